# revision 30
# baseline (speedup 1.0000x reference)
"""MemN2N kernel for 8 Trainium2 NeuronCores.

Math note: in the reference, the attention weights p = mem_mask do not depend
on the query, so every hop adds the same x @ W.  The whole module collapses to

    lengths[b] = sum(masking[b])
    query0[b]  = sentences[b, lengths[b]-1]
    x[b]       = sum_{s < lengths[b]-1} sentences[b, s, :]
    out        = query0 + hops * (x @ W)          # [B, 1, D]

The memory-bound part is the masked row-sum x.  Sharding: batches are
bin-packed 8-per-core (balanced by valid-row count); the host packs only the
valid rows of each batch (padded with zero rows to a 256 multiple) into a flat
row stream per core plus a one-hot row->slot selector, so each core's
TensorEngine computes all 8 of its batch sums in a single PSUM accumulation
chain of float32r matmuls:

    x_ps[8, 512] += sel2[128, 8].T @ a_pair[128, 512]     (1 cyc/row, fp22)

where a_pair holds two 128-row chunks side by side (so each matmul streams 512
columns) and x_ps keeps two half-sums that are added at the end.  Data is
DMA'd in 1 MB tiles alternating between the two HWDGE engines (sync/scalar) to
saturate HBM.  Tail (tiny): transpose x via the PE, two matmuls with (hops*W),
add the query rows, DMA out [8, D] per core.
"""

import math

import numpy as np

import concourse.bass as bass
import concourse.mybir as mybir
from concourse import bacc
from concourse.bass_utils import run_bass_kernel_spmd
from concourse.tile import TileContext

N_CORES = 8
SLOTS = 8  # batches per core
P = 128  # SBUF partitions / rows per chunk
D = 256  # model dim (hardcoded for this problem)
PAIR = 2 * P  # rows per matmul (two chunks side by side)
CPT = 8  # chunks per DMA tile
TILE_ROWS = CPT * P  # 1024 rows = 1 MB per DMA
PPT = CPT // 2  # pair-blocks (= matmuls) per DMA tile

_nc_cache: dict = {}


def _build_bass(T: int, rem: int):
    """Bass program for one core: T DMA tiles of 1024 packed rows plus an
    optional remainder tile of `rem` pair-blocks (256 rows each)."""
    f32 = mybir.dt.float32
    f32r = mybir.dt.float32r
    T2 = T * PPT + rem  # pair blocks

    nc = bacc.Bacc(None)
    a_d = nc.dram_tensor("a", [T, P, CPT * D], f32r, kind="ExternalInput")
    if rem:
        ar_d = nc.dram_tensor("ar", [P, rem * 2 * D], f32r, kind="ExternalInput")
    sel_d = nc.dram_tensor("sel", [P, T2 * SLOTS], f32r, kind="ExternalInput")
    q_d = nc.dram_tensor("q", [SLOTS, D], f32, kind="ExternalInput")
    w_d = nc.dram_tensor("w", [2, P, D], f32, kind="ExternalInput")
    id_d = nc.dram_tensor("id8", [SLOTS, SLOTS], f32, kind="ExternalInput")
    out_d = nc.dram_tensor("out", [SLOTS, D], f32, kind="ExternalOutput")

    with TileContext(nc) as tc:
        with (
            tc.tile_pool(name="const", bufs=1) as cpool,
            tc.tile_pool(name="a", bufs=10) as apool,
            tc.tile_pool(name="acc", bufs=1, space=bass.MemorySpace.PSUM) as accpool,
            tc.tile_pool(name="ps2", bufs=2, space=bass.MemorySpace.PSUM) as ps2pool,
            tc.tile_pool(name="tail", bufs=1) as tpool,
        ):
            # sel + tail constants on the scalar queue so the first a-tile
            # DMAs start immediately on the sync queue
            sel_sb = cpool.tile([P, T2 * SLOTS], f32r)
            nc.scalar.dma_start(out=sel_sb[:], in_=sel_d[:])
            w_sb = cpool.tile([P, 2 * D], f32)
            for h in range(2):
                nc.scalar.dma_start(out=w_sb[:, h * D : (h + 1) * D], in_=w_d[h])
            q_sb = cpool.tile([SLOTS, D], f32)
            nc.scalar.dma_start(out=q_sb[:], in_=q_d[:])
            id_sb = cpool.tile([SLOTS, SLOTS], f32)
            nc.scalar.dma_start(out=id_sb[:], in_=id_d[:])

            # Consume the sel DMA's semaphore with a throwaway PE matmul so
            # loop matmuls don't accumulate extra sync waits.
            warm_ps = ps2pool.tile([SLOTS, SLOTS], f32, tag="warm")
            nc.tensor.matmul(
                warm_ps[:],
                lhsT=sel_sb[:, 0:SLOTS],
                rhs=sel_sb[:, 0:SLOTS],
                start=True,
                stop=True,
            )

            # ---- masked row-sum: x_ps[slot, 0:256/256:512] = even/odd chunk
            # half-sums over all packed rows ----
            x_ps = accpool.tile([SLOTS, 2 * D], f32)
            for t in range(T):
                a_sb = apool.tile([P, CPT * D], f32r)
                eng = nc.sync if t % 2 == 0 else nc.scalar
                eng.dma_start(out=a_sb[:], in_=a_d[t])
                for g in range(PPT):
                    k2 = t * PPT + g
                    nc.tensor.matmul(
                        x_ps[:],
                        lhsT=sel_sb[:, k2 * SLOTS : (k2 + 1) * SLOTS],
                        rhs=a_sb[:, g * 2 * D : (g + 1) * 2 * D],
                        start=(k2 == 0),
                        stop=(k2 == T2 - 1),
                    )
            if rem:
                ar_sb = apool.tile([P, rem * 2 * D], f32r, tag="ar")
                (nc.sync if T % 2 == 0 else nc.scalar).dma_start(
                    out=ar_sb[:], in_=ar_d[:]
                )
                for g in range(rem):
                    k2 = T * PPT + g
                    nc.tensor.matmul(
                        x_ps[:],
                        lhsT=sel_sb[:, k2 * SLOTS : (k2 + 1) * SLOTS],
                        rhs=ar_sb[:, g * 2 * D : (g + 1) * 2 * D],
                        start=(k2 == 0),
                        stop=(k2 == T2 - 1),
                    )

            # ---- tail: out = q + x @ (hops*W) ----
            x_sb = tpool.tile([SLOTS, D], f32)
            nc.vector.tensor_copy(out=x_sb[:], in_=x_ps[:, 0:D])
            nc.vector.tensor_add(out=x_sb[:], in0=x_sb[:], in1=x_ps[:, D : 2 * D])
            xT_sb = tpool.tile([P, 2 * SLOTS], f32)
            for h in range(2):
                tp_ps = ps2pool.tile([P, SLOTS], f32)
                nc.tensor.transpose(tp_ps[:], x_sb[:, h * P : (h + 1) * P], id_sb[:])
                nc.vector.tensor_copy(
                    out=xT_sb[:, h * SLOTS : (h + 1) * SLOTS], in_=tp_ps[:]
                )
            out_ps = ps2pool.tile([SLOTS, D], f32)
            for h in range(2):
                nc.tensor.matmul(
                    out_ps[:],
                    lhsT=xT_sb[:, h * SLOTS : (h + 1) * SLOTS],
                    rhs=w_sb[:, h * D : (h + 1) * D],
                    start=(h == 0),
                    stop=(h == 1),
                )
            out_sb = tpool.tile([SLOTS, D], f32)
            nc.vector.tensor_add(out=out_sb[:], in0=q_sb[:], in1=out_ps[:])
            nc.sync.dma_start(out=out_d[:], in_=out_sb[:])

    nc.compile()  # bacc legalization: splits >1-wait instructions etc.
    return nc


def _prepare(sentences, masking, W, hops):
    """Host-side sharding: lengths, query gather, bin-packing, row packing."""
    sentences = np.ascontiguousarray(np.asarray(sentences), dtype=np.float32)
    masking = np.asarray(masking)
    W = np.ascontiguousarray(np.asarray(W), dtype=np.float32)
    hops = int(np.asarray(hops))

    B, S, Dd = sentences.shape
    assert Dd == D and B % N_CORES == 0
    lengths = masking.astype(np.int64).sum(axis=-1)  # [B]
    qidx = np.clip(lengths - 1, 0, S - 1)
    query = sentences[np.arange(B), qidx]  # [B, D]
    mem_len = np.clip(lengths - 1, 0, S).astype(np.int64)  # valid memory rows
    # pad each batch's row block to a PAIR multiple so every pair-block
    # belongs to exactly one batch (uniform selector; pad rows are zero data)
    padded = ((mem_len + PAIR - 1) // PAIR) * PAIR

    # Bin-pack batches: exactly SLOTS per core, balancing sum(padded) (LPT).
    order = np.argsort(-padded, kind="stable")
    core_load = [0] * N_CORES
    core_batches: list[list[int]] = [[] for _ in range(N_CORES)]
    for b in order:
        open_cores = [c for c in range(N_CORES) if len(core_batches[c]) < SLOTS]
        c = min(open_cores, key=lambda c: core_load[c])
        core_batches[c].append(int(b))
        core_load[c] += int(padded[b])

    # T full 1MB tiles plus a 256-row-granular remainder tile to avoid
    # rounding every core up to a full extra MB
    max_load = max(core_load)
    if max_load <= TILE_ROWS:
        T, rem = 1, 0
    else:
        T = max_load // TILE_ROWS
        rem = (max_load - T * TILE_ROWS + PAIR - 1) // PAIR
    R = T * TILE_ROWS + rem * PAIR
    T2 = R // PAIR

    # fold the hop count into W: out = q + hops * (x @ W) = q + x @ (hops*W)
    w_split = (W * np.float32(hops)).reshape(2, P, D)
    id8 = np.eye(SLOTS, dtype=np.float32)
    in_maps = []
    for c in range(N_CORES):
        A = np.zeros((R, D), dtype=np.float32)
        sel2 = np.zeros((T2, SLOTS), dtype=np.float32)
        pos = 0
        for j, b in enumerate(core_batches[c]):
            m = int(mem_len[b])
            pp = int(padded[b])
            if m > 0:
                A[pos : pos + m] = sentences[b, :m]
                sel2[pos // PAIR : (pos + pp) // PAIR, j] = 1.0
            pos += pp
        # device expects tile t, partition p, chunk cc: row t*TILE_ROWS+cc*P+p
        Afull = A[: T * TILE_ROWS]
        a_dev = np.ascontiguousarray(
            Afull.reshape(T, CPT, P, D).transpose(0, 2, 1, 3).reshape(T, P, CPT * D)
        )
        sel_dev = np.ascontiguousarray(
            np.broadcast_to(sel2.reshape(1, T2 * SLOTS), (P, T2 * SLOTS))
        )
        im = {
            "a": a_dev,
            "sel": sel_dev,
            "q": np.ascontiguousarray(query[core_batches[c]]),
            "w": w_split,
            "id8": id8,
        }
        if rem:
            Ar = A[T * TILE_ROWS :]
            im["ar"] = np.ascontiguousarray(
                Ar.reshape(rem * 2, P, D).transpose(1, 0, 2).reshape(P, rem * 2 * D)
            )
        in_maps.append(im)
    return in_maps, core_batches, (T, rem), hops, B


def _run(sentences, masking, W, hops, trace=False):
    in_maps, core_batches, key, hops_i, B = _prepare(sentences, masking, W, hops)
    if key not in _nc_cache:
        _nc_cache[key] = _build_bass(*key)
    nc = _nc_cache[key]
    res = run_bass_kernel_spmd(
        nc, in_maps, core_ids=list(range(N_CORES)), trace=trace
    )
    out = np.empty((B, 1, D), dtype=np.float32)
    for c in range(N_CORES):
        r = res.results[c]["out"]
        for j, b in enumerate(core_batches[c]):
            out[b, 0] = r[j]
    return out, res


def kernel(sentences, masking, W, hops):
    out, _ = _run(sentences, masking, W, hops)
    return out


# revision 31
# speedup vs baseline: 1.4945x; 1.4945x over previous
"""MemN2N kernel for 8 Trainium2 NeuronCores.

Math note: in the reference, the attention weights p = mem_mask do not depend
on the query, so every hop adds the same x @ W.  The whole module collapses to

    lengths[b] = sum(masking[b])
    query0[b]  = sentences[b, lengths[b]-1]
    x[b]       = sum_{s < lengths[b]-1} sentences[b, s, :]
    out        = query0 + hops * (x @ W)          # [B, 1, D]

The memory-bound part is the masked row-sum x.  Sharding: batches are
bin-packed 8-per-core (balanced by valid-row count); the host packs only the
valid rows of each batch (padded with zero rows to a 256 multiple) into a flat
row stream per core plus a one-hot row->slot selector, so each core's
TensorEngine computes all 8 of its batch sums in a single PSUM accumulation
chain of float32r matmuls:

    x_ps[8, 512] += sel2[128, 8].T @ a_pair[128, 512]     (1 cyc/row, fp22)

where a_pair holds two 128-row chunks side by side (so each matmul streams 512
columns) and x_ps keeps two half-sums that are added at the end.  Data is
DMA'd in 1 MB tiles alternating between the two HWDGE engines (sync/scalar) to
saturate HBM, plus one 256-row-granular remainder tile so cores don't round up
to a full extra MB.  Tail (tiny): transpose x via the PE, two matmuls with
(hops*W), add the query rows, DMA out [8, D] per core.
"""

import math

import numpy as np

import concourse.bass as bass
import concourse.mybir as mybir
from concourse import bacc
from concourse.bass_utils import run_bass_kernel_spmd
from concourse.tile import TileContext

N_CORES = 8
SLOTS = 8  # batches per core
P = 128  # SBUF partitions / rows per chunk
D = 256  # model dim (hardcoded for this problem)
PAIR = 2 * P  # rows per matmul (two chunks side by side)
CPT = 8  # chunks per DMA tile
TILE_ROWS = CPT * P  # 1024 rows = 1 MB per DMA
PPT = CPT // 2  # pair-blocks (= matmuls) per DMA tile

_nc_cache: dict = {}


def _build_bass(T: int, rem: int):
    """Bass program for one core: T DMA tiles of 1024 packed rows plus an
    optional remainder tile of `rem` pair-blocks (256 rows each)."""
    f32 = mybir.dt.float32
    f32r = mybir.dt.float32r
    T2 = T * PPT + rem  # pair blocks

    nc = bacc.Bacc(None)
    a_d = nc.dram_tensor("a", [T, P, CPT * D], f32r, kind="ExternalInput")
    if rem:
        ar_d = nc.dram_tensor("ar", [P, rem * 2 * D], f32r, kind="ExternalInput")
    sel_d = nc.dram_tensor("sel", [P, T2 * SLOTS], f32r, kind="ExternalInput")
    q_d = nc.dram_tensor("q", [SLOTS, D], f32, kind="ExternalInput")
    w_d = nc.dram_tensor("w", [2, P, D], f32, kind="ExternalInput")
    id_d = nc.dram_tensor("id8", [SLOTS, SLOTS], f32, kind="ExternalInput")
    out_d = nc.dram_tensor("out", [SLOTS, D], f32, kind="ExternalOutput")

    with TileContext(nc) as tc:
        with (
            tc.tile_pool(name="const", bufs=1) as cpool,
            tc.tile_pool(name="a", bufs=10) as apool,
            tc.tile_pool(name="acc", bufs=1, space=bass.MemorySpace.PSUM) as accpool,
            tc.tile_pool(name="ps2", bufs=2, space=bass.MemorySpace.PSUM) as ps2pool,
            tc.tile_pool(name="tail", bufs=1) as tpool,
        ):
            # sel + tail constants on the scalar queue so the first a-tile
            # DMAs start immediately on the sync queue
            sel_sb = cpool.tile([P, T2 * SLOTS], f32r)
            nc.scalar.dma_start(out=sel_sb[:], in_=sel_d[:])
            w_sb = cpool.tile([P, 2 * D], f32)
            for h in range(2):
                nc.scalar.dma_start(out=w_sb[:, h * D : (h + 1) * D], in_=w_d[h])
            q_sb = cpool.tile([SLOTS, D], f32)
            nc.scalar.dma_start(out=q_sb[:], in_=q_d[:])
            id_sb = cpool.tile([SLOTS, SLOTS], f32)
            nc.scalar.dma_start(out=id_sb[:], in_=id_d[:])

            # Consume the sel DMA's semaphore with a throwaway PE matmul so
            # loop matmuls don't accumulate extra sync waits.
            warm_ps = ps2pool.tile([SLOTS, SLOTS], f32, tag="warm")
            nc.tensor.matmul(
                warm_ps[:],
                lhsT=sel_sb[:, 0:SLOTS],
                rhs=sel_sb[:, 0:SLOTS],
                start=True,
                stop=True,
            )

            # ---- masked row-sum: x_ps[slot, 0:256/256:512] = even/odd chunk
            # half-sums over all packed rows ----
            x_ps = accpool.tile([SLOTS, 2 * D], f32)
            for t in range(T):
                a_sb = apool.tile([P, CPT * D], f32r)
                eng = nc.sync if t % 2 == 0 else nc.scalar
                eng.dma_start(out=a_sb[:], in_=a_d[t])
                for g in range(PPT):
                    k2 = t * PPT + g
                    nc.tensor.matmul(
                        x_ps[:],
                        lhsT=sel_sb[:, k2 * SLOTS : (k2 + 1) * SLOTS],
                        rhs=a_sb[:, g * 2 * D : (g + 1) * 2 * D],
                        start=(k2 == 0),
                        stop=(k2 == T2 - 1),
                    )
            if rem:
                ar_sb = apool.tile([P, rem * 2 * D], f32r, tag="ar")
                (nc.sync if T % 2 == 0 else nc.scalar).dma_start(
                    out=ar_sb[:], in_=ar_d[:]
                )
                for g in range(rem):
                    k2 = T * PPT + g
                    nc.tensor.matmul(
                        x_ps[:],
                        lhsT=sel_sb[:, k2 * SLOTS : (k2 + 1) * SLOTS],
                        rhs=ar_sb[:, g * 2 * D : (g + 1) * 2 * D],
                        start=(k2 == 0),
                        stop=(k2 == T2 - 1),
                    )

            # ---- tail: out = q + x @ (hops*W) ----
            x_sb = tpool.tile([SLOTS, D], f32)
            nc.vector.tensor_copy(out=x_sb[:], in_=x_ps[:, 0:D])
            nc.vector.tensor_add(out=x_sb[:], in0=x_sb[:], in1=x_ps[:, D : 2 * D])
            xT_sb = tpool.tile([P, 2 * SLOTS], f32)
            for h in range(2):
                tp_ps = ps2pool.tile([P, SLOTS], f32)
                nc.tensor.transpose(tp_ps[:], x_sb[:, h * P : (h + 1) * P], id_sb[:])
                nc.vector.tensor_copy(
                    out=xT_sb[:, h * SLOTS : (h + 1) * SLOTS], in_=tp_ps[:]
                )
            out_ps = ps2pool.tile([SLOTS, D], f32)
            for h in range(2):
                nc.tensor.matmul(
                    out_ps[:],
                    lhsT=xT_sb[:, h * SLOTS : (h + 1) * SLOTS],
                    rhs=w_sb[:, h * D : (h + 1) * D],
                    start=(h == 0),
                    stop=(h == 1),
                )
            out_sb = tpool.tile([SLOTS, D], f32)
            nc.vector.tensor_add(out=out_sb[:], in0=q_sb[:], in1=out_ps[:])
            nc.sync.dma_start(out=out_d[:], in_=out_sb[:])

    nc.compile()  # bacc legalization: splits >1-wait instructions etc.
    return nc


def _prepare(sentences, masking, W, hops):
    """Host-side sharding: lengths, query gather, bin-packing, row packing."""
    sentences = np.ascontiguousarray(np.asarray(sentences), dtype=np.float32)
    masking = np.asarray(masking)
    W = np.ascontiguousarray(np.asarray(W), dtype=np.float32)
    hops = int(np.asarray(hops))

    B, S, Dd = sentences.shape
    assert Dd == D and B % N_CORES == 0
    lengths = masking.astype(np.int64).sum(axis=-1)  # [B]
    qidx = np.clip(lengths - 1, 0, S - 1)
    query = sentences[np.arange(B), qidx]  # [B, D]
    mem_len = np.clip(lengths - 1, 0, S).astype(np.int64)  # valid memory rows
    # pad each batch's row block to a PAIR multiple so every pair-block
    # belongs to exactly one batch (uniform selector; pad rows are zero data)
    padded = ((mem_len + PAIR - 1) // PAIR) * PAIR

    # Bin-pack batches: exactly SLOTS per core, balancing sum(padded) (LPT).
    order = np.argsort(-padded, kind="stable")
    core_load = [0] * N_CORES
    core_batches: list[list[int]] = [[] for _ in range(N_CORES)]
    for b in order:
        open_cores = [c for c in range(N_CORES) if len(core_batches[c]) < SLOTS]
        c = min(open_cores, key=lambda c: core_load[c])
        core_batches[c].append(int(b))
        core_load[c] += int(padded[b])

    # T full 1MB tiles plus a 256-row-granular remainder tile to avoid
    # rounding every core up to a full extra MB
    max_load = max(core_load)
    if max_load <= TILE_ROWS:
        T, rem = 1, 0
    else:
        T = max_load // TILE_ROWS
        rem = (max_load - T * TILE_ROWS + PAIR - 1) // PAIR
    R = T * TILE_ROWS + rem * PAIR
    T2 = R // PAIR

    # fold the hop count into W: out = q + hops * (x @ W) = q + x @ (hops*W)
    w_split = (W * np.float32(hops)).reshape(2, P, D)
    id8 = np.eye(SLOTS, dtype=np.float32)
    in_maps = []
    for c in range(N_CORES):
        A = np.zeros((R, D), dtype=np.float32)
        sel2 = np.zeros((T2, SLOTS), dtype=np.float32)
        pos = 0
        for j, b in enumerate(core_batches[c]):
            m = int(mem_len[b])
            pp = int(padded[b])
            if m > 0:
                A[pos : pos + m] = sentences[b, :m]
                sel2[pos // PAIR : (pos + pp) // PAIR, j] = 1.0
            pos += pp
        # device expects tile t, partition p, chunk cc: row t*TILE_ROWS+cc*P+p
        Afull = A[: T * TILE_ROWS]
        a_dev = np.ascontiguousarray(
            Afull.reshape(T, CPT, P, D).transpose(0, 2, 1, 3).reshape(T, P, CPT * D)
        )
        sel_dev = np.ascontiguousarray(
            np.broadcast_to(sel2.reshape(1, T2 * SLOTS), (P, T2 * SLOTS))
        )
        im = {
            "a": a_dev,
            "sel": sel_dev,
            "q": np.ascontiguousarray(query[core_batches[c]]),
            "w": w_split,
            "id8": id8,
        }
        if rem:
            Ar = A[T * TILE_ROWS :]
            im["ar"] = np.ascontiguousarray(
                Ar.reshape(rem * 2, P, D).transpose(1, 0, 2).reshape(P, rem * 2 * D)
            )
        in_maps.append(im)
    return in_maps, core_batches, (T, rem), hops, B


def _run(sentences, masking, W, hops, trace=False):
    in_maps, core_batches, key, hops_i, B = _prepare(sentences, masking, W, hops)
    if key not in _nc_cache:
        _nc_cache[key] = _build_bass(*key)
    nc = _nc_cache[key]
    res = run_bass_kernel_spmd(
        nc, in_maps, core_ids=list(range(N_CORES)), trace=trace
    )
    out = np.empty((B, 1, D), dtype=np.float32)
    for c in range(N_CORES):
        r = res.results[c]["out"]
        for j, b in enumerate(core_batches[c]):
            out[b, 0] = r[j]
    return out, res


def kernel(sentences, masking, W, hops):
    out, _ = _run(sentences, masking, W, hops)
    return out
